# revision 4
# baseline (speedup 1.0000x reference)
"""MultiHeadGlobalAttention (segment softmax attention pooling) on 8 trn2 cores, v2.

Sharding: 128 segments/core (batch sorted -> contiguous node ranges), weights
replicated. Per-core blocks of 4096 nodes = 4 strips x 1024; macro = 512 nodes
(128 node-columns x 4 strips packed into the fp8 DoubleRow contraction grid of
128 partitions x 2 pairs).

Device pipeline per macro:
  gate1(h, 8 mm) + mlp1(f1, 4 mm) as fp8e4 DoubleRow (block-"diagonal" strip
  weights); |h| evac split ACT/DVE/Pool -> bf16; f1 evac (bias+relu) on Pool
  -> bf16; gate2 as plain bf16 accumulating matmuls; exp on ACT; e = e0*eg
  (host linear-gate exp) on Pool; mlp2 plain bf16 data-stationary; y = e*f2 on
  DVE -> fp8; e concat on Pool; segment-reduce via host-DMA'd fp8 indicator in
  DoubleRow chunk-pairs into resident PSUM acc [128 segs, 132]; final divide +
  output bias on DVE.
"""

import sys

for _p in ("/opt/trn_rl_repo", "/root/.axon_site/_ro/trn_rl_repo"):
    if _p not in sys.path:
        sys.path.append(_p)

import numpy as np

IN_CH = 64
OUT_CH = 32
HEADS = 4
NUM_SEGS = 1024
N_CORES = 8
SEGS_PER_CORE = NUM_SEGS // N_CORES  # 128

P = 128
BLK_NODES = 4096
STRIP = 1024            # nodes per strip per block (4 strips)
MACROS_PER_BLK = 8      # each macro: 128 node-cols x 4 strips = 512 nodes
ML = P + HEADS          # 132: 128 feat cols + 4 e cols

# habs evac split (flat cols of [128, 1024]): ACT / DVE / Pool
HABS_ACT = 1024
HABS_DVE = 0
F1_ON_ACT = False
F1_ACT_COLS = 0
IND_DMA_ACT = False


def _build_bass(Mpad):
    import concourse.bacc as bacc
    import concourse.tile as tile
    from concourse import mybir

    fp32 = mybir.dt.float32
    bf16 = mybir.dt.bfloat16
    fp8 = mybir.dt.float8e4
    AF = mybir.ActivationFunctionType
    ALU = mybir.AluOpType
    DR = mybir.MatmulPerfMode.DoubleRow

    nc = bacc.Bacc("TRN2", target_bir_lowering=False, debug=False)

    xq_d = nc.dram_tensor("xq", [P, Mpad // 2], fp8, kind="ExternalInput")
    ind_d = nc.dram_tensor("ind", [P, Mpad], fp8, kind="ExternalInput")
    w12z_d = nc.dram_tensor("w12z", [P, 2 * 4 * HEADS], bf16,
                            kind="ExternalInput")
    wh_d = nc.dram_tensor("wh", [P, 2 * 8 * P], fp8, kind="ExternalInput")
    wm_d = nc.dram_tensor("wm", [P, 2 * 4 * P], fp8, kind="ExternalInput")
    w2h_d = nc.dram_tensor("w2h", [P, 2 * HEADS], bf16, kind="ExternalInput")
    m2_d = nc.dram_tensor("m2t", [P, P], bf16, kind="ExternalInput")
    b1_d = nc.dram_tensor("b1", [P, 1], fp32, kind="ExternalInput")
    b2_d = nc.dram_tensor("b2r", [P, P], fp32, kind="ExternalInput")
    out_d = nc.dram_tensor("out", [P, P], fp32, kind="ExternalOutput")

    assert Mpad % 2048 == 0
    blocks = [BLK_NODES] * (Mpad // BLK_NODES)
    if Mpad % BLK_NODES:
        blocks.append(Mpad % BLK_NODES)

    with tile.TileContext(nc) as tc:
        with (
            tc.tile_pool(name="const", bufs=1) as cpool,
            tc.tile_pool(name="xin", bufs=2) as xpool,
            tc.tile_pool(name="hsb", bufs=2) as hpool,
            tc.tile_pool(name="fsb", bufs=2) as fpool,
            tc.tile_pool(name="ysb", bufs=2) as ypool,
            tc.tile_pool(name="esb", bufs=2) as epool,
            tc.tile_pool(name="ps_h", bufs=2, space="PSUM") as ps_h,
            tc.tile_pool(name="ps_f1", bufs=1, space="PSUM") as ps_f1,
            tc.tile_pool(name="ps_f", bufs=1, space="PSUM") as ps_f,
            tc.tile_pool(name="ps_misc", bufs=1, space="PSUM") as ps_misc,
        ):
            # ---- static setup ----
            wh_sb = cpool.tile([P, 2 * 8 * P], fp8)
            nc.sync.dma_start(out=wh_sb[:], in_=wh_d[:])
            wm_sb = cpool.tile([P, 2 * 4 * P], fp8)
            nc.sync.dma_start(out=wm_sb[:], in_=wm_d[:])
            w2h_sb = cpool.tile([P, 2 * HEADS], bf16)
            nc.sync.dma_start(out=w2h_sb[:], in_=w2h_d[:])
            m2_sb = cpool.tile([P, P], bf16)
            nc.sync.dma_start(out=m2_sb[:], in_=m2_d[:])
            b1_sb = cpool.tile([P, 1], fp32)
            nc.sync.dma_start(out=b1_sb[:], in_=b1_d[:])
            b2_sb = cpool.tile([P, P], fp32)
            nc.sync.dma_start(out=b2_sb[:], in_=b2_d[:])

            w12z_sb = cpool.tile([P, 2 * 4 * HEADS], bf16)
            nc.sync.dma_start(out=w12z_sb[:], in_=w12z_d[:])

            wh4 = wh_sb[:].rearrange("p (i k m) -> p i k m", i=2, k=8)
            wm4 = wm_sb[:].rearrange("p (i s m) -> p i s m", i=2, s=4)
            w2q = w2h_sb[:].rearrange("p (q h) -> p q h", q=2)
            w12z4 = w12z_sb[:].rearrange("p (i c h) -> p i c h", i=2, c=4)

            acc_ps = ps_misc.tile([P, ML], fp32, tag="acc")

            n_macros = Mpad // 512
            seg_state = {"n": 0}

            def tail_a(ctx):
                """gate2(+glin fold) -> exp -> mlp2 -> y for macro ctx."""
                habs, f1_sb, m = ctx["habs"], ctx["f1"], ctx["m"]
                xq3, mo = ctx["xq3"], ctx["m"] * P
                g_ps_t = ps_misc.tile([P, 16], fp32, tag="gps")
                g_ps = g_ps_t[:]
                for c in range(4):
                    gc = g_ps[:, c * 4:(c + 1) * 4]
                    for q in range(2):
                        nc.tensor.matmul(
                            out=gc,
                            lhsT=habs[:, q * 512 + c * P:q * 512 + (c + 1) * P],
                            rhs=w2q[:, q, :],
                            start=(q == 0), stop=False)
                    # + linear gate part: c_lin * (W2 W1) x, from resident x
                    for i in range(2):
                        nc.tensor.matmul(
                            out=gc, lhsT=xq3[:, i, mo:mo + P],
                            rhs=w12z4[:, i, c, :],
                            start=False, stop=(i == 1))
                e_sb = epool.tile([P, 16], fp32, tag="esb")
                nc.scalar.activation(e_sb[:], g_ps, AF.Exp)
                f_ps = ps_f.tile([P, 512], fp32, tag="fps")
                for c in range(4):
                    nc.tensor.matmul(
                        out=f_ps[:, c * P:(c + 1) * P],
                        lhsT=f1_sb[:, c * P:(c + 1) * P],
                        rhs=m2_sb[:], start=True, stop=True)
                y_sb = ypool.tile([P, 2 * 2 * ML], fp8, tag="ysb")
                y4 = y_sb[:].rearrange("p (j i f) -> p j i f", j=2, i=2)
                nc.vector.tensor_tensor(
                    out=y4[:, :, :, 0:P].rearrange(
                        "p j i (h o) -> p j i h o", h=HEADS),
                    in0=f_ps[:].rearrange("p (j i h o) -> p j i h o",
                                          j=2, i=2, h=HEADS),
                    in1=e_sb[:].rearrange("p (j i h) -> p j i h", j=2, i=2)
                        .unsqueeze(4).broadcast_to([P, 2, 2, HEADS, OUT_CH]),
                    op=ALU.mult)
                nc.gpsimd.tensor_copy(
                    out=y4[:, :, :, P:ML],
                    in_=e_sb[:].rearrange("p (j i h) -> p j i h", j=2, i=2))
                ctx["y4"] = y4

            def tail_b(ctx):
                """segment accumulate for macro ctx (2 behind)."""
                ind5, m, y4 = ctx["ind5"], ctx["m"], ctx["y4"]
                for j in range(2):
                    nc.tensor.matmul(
                        out=acc_ps[:],
                        lhsT=ind5[:, m, j, :, :],
                        rhs=y4[:, j, :, :],
                        start=(seg_state["n"] == 0),
                        stop=(seg_state["n"] == 2 * n_macros - 1),
                        perf_mode=DR)
                    seg_state["n"] += 1

            pend = []
            boff = 0
            for bn in blocks:
                strip_b = bn // 4
                macros_b = bn // 512
                xq = xpool.tile([P, 2 * strip_b], fp8, tag="xq")
                nc.sync.dma_start(
                    out=xq[:], in_=xq_d[:, boff // 2:boff // 2 + 2 * strip_b])
                ind = xpool.tile([P, bn], fp8, tag="ind")
                nc.sync.dma_start(out=ind[:], in_=ind_d[:, boff:boff + bn])
                boff += bn
                xq3 = xq[:].rearrange("p (i n) -> p i n", i=2)
                ind5 = ind[:].rearrange("p (m j i s) -> p m j i s",
                                        m=macros_b, j=2, i=2)

                for m in range(macros_b):
                    mo = m * P
                    rhs = xq3[:, :, mo:mo + P]
                    # ---- gate1 + mlp1 (fp8 DoubleRow) ----
                    h_ps = ps_h.tile([P, 1024], fp32, tag="hps")
                    h4 = h_ps[:].rearrange("p (q s n) -> p q s n", q=2, s=4)
                    f1_ps = ps_f1.tile([P, 512], fp32, tag="f1ps")
                    f13 = f1_ps[:].rearrange("p (s n) -> p s n", s=4)
                    for s in range(4):
                        for q in range(2):
                            nc.tensor.matmul(
                                out=h4[:, q, s, :], lhsT=wh4[:, :, s * 2 + q, :],
                                rhs=rhs, start=True, stop=True, perf_mode=DR)
                        nc.tensor.matmul(
                            out=f13[:, s, :], lhsT=wm4[:, :, s, :],
                            rhs=rhs, start=True, stop=True, perf_mode=DR)

                    # ---- habs evac split across ACT / DVE / Pool ----
                    # (GPSIMD cannot read PSUM on real hw: evacs are ACT/DVE)
                    habs = hpool.tile([P, 1024], bf16, tag="habs")
                    c0 = HABS_ACT
                    nc.scalar.activation(habs[:, 0:c0], h_ps[:, 0:c0], AF.Abs)
                    if c0 < 1024:
                        nc.vector.tensor_scalar(
                            out=habs[:, c0:1024], in0=h_ps[:, c0:1024],
                            scalar1=0.0, scalar2=None, op0=ALU.abs_max)

                    # ---- f1 evac (bias + relu) on DVE ----
                    f1_sb = fpool.tile([P, 512], bf16, tag="f1sb")
                    fa = F1_ACT_COLS
                    if fa:
                        nc.scalar.activation(
                            f1_sb[:, 0:fa], f1_ps[:, 0:fa], AF.Relu,
                            bias=b1_sb[:, 0:1])
                    if fa < 512:
                        nc.vector.tensor_scalar(
                            out=f1_sb[:, fa:512], in0=f1_ps[:, fa:512],
                            scalar1=b1_sb[:, 0:1], scalar2=0.0,
                            op0=ALU.add, op1=ALU.max)

                    # ---- software-pipelined tails: a(m-1), b(m-2) ----
                    pend.append({"habs": habs, "f1": f1_sb, "xq3": xq3,
                                 "ind5": ind5, "m": m})
                    if len(pend) >= 2:
                        tail_a(pend[-2])
                    if len(pend) >= 3:
                        tail_b(pend[0])
                        pend.pop(0)

            # drain pipeline
            tail_a(pend[-1])
            for ctx in pend:
                tail_b(ctx)

            # ---- final: out = num/den + b2 ----
            den_sb = cpool.tile([P, HEADS], fp32)
            nc.vector.tensor_scalar(
                out=den_sb[:], in0=acc_ps[:, P:ML], scalar1=1e-16,
                scalar2=None, op0=ALU.add)
            rec_sb = cpool.tile([P, HEADS], fp32)
            nc.vector.reciprocal(rec_sb[:], den_sb[:])
            out_sb = cpool.tile([P, P], fp32)
            nc.vector.tensor_tensor(
                out=out_sb[:].rearrange("p (h o) -> p h o", h=HEADS),
                in0=acc_ps[:, 0:P].rearrange("p (h o) -> p h o", h=HEADS),
                in1=rec_sb[:].unsqueeze(2).broadcast_to([P, HEADS, OUT_CH]),
                op=ALU.mult)
            nc.vector.tensor_tensor(
                out=out_sb[:], in0=out_sb[:], in1=b2_sb[:], op=ALU.add)
            nc.sync.dma_start(out=out_d[:], in_=out_sb[:])

    nc.compile()
    return nc


def _host_inputs(x, batch, gate_w1, prelu_a, gate_w2, mlp_w1, mlp_b1,
                 mlp_w2, mlp_b2, bnds, Mpad):
    """Build shared weight arrays + per-core input maps."""
    import ml_dtypes
    f8 = ml_dtypes.float8_e4m3
    bf = ml_dtypes.bfloat16

    a = float(np.asarray(prelu_a))
    c_abs = (1.0 - a) / 2.0
    c_lin = (1.0 + a) / 2.0

    # fp8 weights scaled x16 (away from subnormals); unscaled exactly via the
    # bf16 second-layer weights (w2h, m2t) and b1.
    WS = 16.0
    # wh[p, i, s*2+q, m] = [strip(p,i)==s] * WS*gate_w1[q*128+m, p%64]
    wh = np.zeros((P, 2, 8, P), np.float32)
    wm = np.zeros((P, 2, 4, P), np.float32)
    g1 = np.asarray(gate_w1, np.float32)      # [256, 64]
    m1 = np.asarray(mlp_w1, np.float32)       # [128, 64]
    for s in range(4):
        ph, i = s % 2, s // 2
        for q in range(2):
            wh[64 * ph:64 * (ph + 1), i, s * 2 + q, :] = \
                WS * g1[q * P:(q + 1) * P, :].T
        wm[64 * ph:64 * (ph + 1), i, s, :] = WS * m1.T
    w2h = (c_abs / WS * np.asarray(gate_w2, np.float32).T)  # [256, 4]
    w2h = w2h.reshape(2, P, HEADS).transpose(1, 0, 2)  # [128, 2, 4]
    # w12z[p, i, c, h] = [strip(p,i)==c] * c_lin * (gate_w2 @ gate_w1).T[ch, h]
    w12 = c_lin * (np.asarray(gate_w2, np.float32) @ g1).T  # [64, 4]
    w12z = np.zeros((P, 2, 4, HEADS), np.float32)
    for c in range(4):
        ph, i = c % 2, c // 2
        w12z[64 * ph:64 * (ph + 1), i, c, :] = w12

    shared = {
        "wh": np.ascontiguousarray(wh.reshape(P, -1)).astype(f8),
        "wm": np.ascontiguousarray(wm.reshape(P, -1)).astype(f8),
        "w2h": np.ascontiguousarray(w2h.reshape(P, -1)).astype(bf),
        "m2t": np.ascontiguousarray(
            np.asarray(mlp_w2, np.float32).T / WS).astype(bf),
        "b1": np.ascontiguousarray(
            WS * np.asarray(mlp_b1, np.float32).reshape(P, 1)),
        "b2r": np.ascontiguousarray(
            np.tile(np.asarray(mlp_b2, np.float32).reshape(1, P), (P, 1))),
        "w12z": np.ascontiguousarray(w12z.reshape(P, -1)).astype(bf),
    }

    blocks = [BLK_NODES] * (Mpad // BLK_NODES)
    if Mpad % BLK_NODES:
        blocks.append(Mpad % BLK_NODES)
    in_maps = []
    for c in range(len(bnds) - 1):
        r0, r1 = int(bnds[c]), int(bnds[c + 1])
        cnt = r1 - r0
        xs = np.zeros((Mpad, IN_CH), np.float32)
        xs[:cnt] = x[r0:r1]
        x8 = xs.astype(f8)
        onehot = np.zeros((Mpad, P), np.uint8)
        bid = (batch[r0:r1] - c * SEGS_PER_CORE).astype(np.int64)
        onehot[np.arange(cnt), bid] = 1
        # per block: xq[p, i*sb + col] = x[node(b, 2i+p//64, col), p%64]
        #            ind[p, (m, c, s)] = onehot[node(b, c, m*128+p), s]
        xq = np.empty((P, Mpad // 2), f8)
        ind = np.empty((P, Mpad), np.uint8)
        noff = 0
        for bn in blocks:
            sb = bn // 4
            xb = x8[noff:noff + bn].reshape(4, sb, IN_CH)
            for i in range(2):
                for ph in range(2):
                    xq[64 * ph:64 * (ph + 1),
                       noff // 2 + i * sb:noff // 2 + (i + 1) * sb] = \
                        xb[2 * i + ph].T
            ib = onehot[noff:noff + bn].reshape(4, bn // 512, P, P)
            ind[:, noff:noff + bn] = \
                ib.transpose(2, 1, 0, 3).reshape(P, bn)
            noff += bn
        in_maps.append({
            "xq": np.ascontiguousarray(xq),
            "ind": np.ascontiguousarray(ind).astype(f8),
            **shared,
        })
    return in_maps


def kernel(x, batch, num_segments, gate_w1, prelu_a, gate_w2,
           mlp_w1, mlp_b1, mlp_w2, mlp_b2):
    from concourse.bass_utils import run_bass_kernel_spmd

    x = np.asarray(x, dtype=np.float32)
    batch = np.asarray(batch, dtype=np.int32)

    bnds = np.searchsorted(batch, np.arange(0, NUM_SEGS + 1, SEGS_PER_CORE))
    counts = np.diff(bnds)
    Mpad = int(-(-counts.max() // 2048) * 2048)

    nc = _build_bass(Mpad)
    in_maps = _host_inputs(x, batch, gate_w1, prelu_a, gate_w2, mlp_w1,
                           mlp_b1, mlp_w2, mlp_b2, bnds, Mpad)
    res = run_bass_kernel_spmd(nc, in_maps, core_ids=list(range(N_CORES)))
    out = np.concatenate([res.results[c]["out"] for c in range(N_CORES)],
                         axis=0)
    return out.astype(np.float32)


# revision 6
# speedup vs baseline: 1.0747x; 1.0747x over previous
"""MultiHeadGlobalAttention (segment softmax attention pooling) on 8 trn2 cores, v2.

Sharding: 128 segments/core (batch sorted -> contiguous node ranges), weights
replicated. Per-core blocks of 4096 nodes = 4 strips x 1024; macro = 512 nodes
(128 node-columns x 4 strips packed into the fp8 DoubleRow contraction grid of
128 partitions x 2 pairs).

Device pipeline per macro (software-pipelined: front(m) | gate2..y(m-1) |
segment-reduce(m-2)):
  gate1(h, 8 mm) + mlp1(f1, 4 mm) as fp8e4 DoubleRow (block-"diagonal" strip
  weights, x16-scaled weights unscaled via the bf16 second-layer weights);
  |h| evac on ACT -> bf16; f1 evac (bias+relu) on DVE -> bf16; gate2 as plain
  bf16 accumulating matmuls with the host linear-gate part (c_lin*W2W1)
  folded in as two extra matmuls against the resident fp8 x; exp on ACT;
  mlp2 plain bf16 data-stationary; y = e*f2 on DVE -> fp8; e concat on Pool
  (SBUF-only: GPSIMD cannot access PSUM); segment-reduce via host-DMA'd fp8
  indicator in DoubleRow chunk-pairs into resident PSUM acc [128 segs, 132];
  final divide + output bias on DVE. Blocks of 4096 nodes (+ optional 2048
  tail block).
"""

import sys

for _p in ("/opt/trn_rl_repo", "/root/.axon_site/_ro/trn_rl_repo"):
    if _p not in sys.path:
        sys.path.append(_p)

import numpy as np

IN_CH = 64
OUT_CH = 32
HEADS = 4
NUM_SEGS = 1024
N_CORES = 8
SEGS_PER_CORE = NUM_SEGS // N_CORES  # 128

P = 128
BLK_NODES = 4096
STRIP = 1024            # nodes per strip per block (4 strips)
MACROS_PER_BLK = 8      # each macro: 128 node-cols x 4 strips = 512 nodes
ML = P + HEADS          # 132: 128 feat cols + 4 e cols

# habs evac split point (flat cols of [128, 1024]): [0:HABS_ACT) on ACT,
# rest on DVE. PSUM evacuation is legal only on ACT/DVE (GPSIMD cannot
# read PSUM on real hardware).
HABS_ACT = 1024


def _build_bass(Mpad):
    import concourse.bacc as bacc
    import concourse.tile as tile
    from concourse import mybir

    fp32 = mybir.dt.float32
    bf16 = mybir.dt.bfloat16
    fp8 = mybir.dt.float8e4
    AF = mybir.ActivationFunctionType
    ALU = mybir.AluOpType
    DR = mybir.MatmulPerfMode.DoubleRow

    nc = bacc.Bacc("TRN2", target_bir_lowering=False, debug=False)

    xq_d = nc.dram_tensor("xq", [P, Mpad // 2], fp8, kind="ExternalInput")
    ind_d = nc.dram_tensor("ind", [P, Mpad], fp8, kind="ExternalInput")
    w12z_d = nc.dram_tensor("w12z", [P, 2 * 4 * HEADS], bf16,
                            kind="ExternalInput")
    wh_d = nc.dram_tensor("wh", [P, 2 * 8 * P], fp8, kind="ExternalInput")
    wm_d = nc.dram_tensor("wm", [P, 2 * 4 * P], fp8, kind="ExternalInput")
    w2h_d = nc.dram_tensor("w2h", [P, 2 * HEADS], bf16, kind="ExternalInput")
    m2_d = nc.dram_tensor("m2t", [P, P], bf16, kind="ExternalInput")
    b1_d = nc.dram_tensor("b1", [P, 1], fp32, kind="ExternalInput")
    b2_d = nc.dram_tensor("b2r", [P, P], fp32, kind="ExternalInput")
    out_d = nc.dram_tensor("out", [P, P], fp32, kind="ExternalOutput")

    assert Mpad % 2048 == 0
    blocks = [BLK_NODES] * (Mpad // BLK_NODES)
    if Mpad % BLK_NODES:
        blocks.append(Mpad % BLK_NODES)

    with tile.TileContext(nc) as tc:
        with (
            tc.tile_pool(name="const", bufs=1) as cpool,
            tc.tile_pool(name="xin", bufs=2) as xpool,
            tc.tile_pool(name="hsb", bufs=4) as hpool,
            tc.tile_pool(name="fsb", bufs=4) as fpool,
            tc.tile_pool(name="ysb", bufs=4) as ypool,
            tc.tile_pool(name="esb", bufs=4) as epool,
            tc.tile_pool(name="ps_h", bufs=2, space="PSUM") as ps_h,
            tc.tile_pool(name="ps_f1", bufs=1, space="PSUM") as ps_f1,
            tc.tile_pool(name="ps_f", bufs=1, space="PSUM") as ps_f,
            tc.tile_pool(name="ps_misc", bufs=1, space="PSUM") as ps_misc,
        ):
            # ---- static setup ----
            wh_sb = cpool.tile([P, 2 * 8 * P], fp8)
            nc.sync.dma_start(out=wh_sb[:], in_=wh_d[:])
            wm_sb = cpool.tile([P, 2 * 4 * P], fp8)
            nc.sync.dma_start(out=wm_sb[:], in_=wm_d[:])
            w2h_sb = cpool.tile([P, 2 * HEADS], bf16)
            nc.sync.dma_start(out=w2h_sb[:], in_=w2h_d[:])
            m2_sb = cpool.tile([P, P], bf16)
            nc.sync.dma_start(out=m2_sb[:], in_=m2_d[:])
            b1_sb = cpool.tile([P, 1], fp32)
            nc.sync.dma_start(out=b1_sb[:], in_=b1_d[:])
            # touch ACT early so the activation-table load (~1.3us)
            # overlaps the first block's input DMAs
            warm_sb = cpool.tile([P, 1], fp32)
            nc.scalar.activation(warm_sb[:], b1_sb[:], AF.Abs)
            b2_sb = cpool.tile([P, P], fp32)
            nc.sync.dma_start(out=b2_sb[:], in_=b2_d[:])

            w12z_sb = cpool.tile([P, 2 * 4 * HEADS], bf16)
            nc.sync.dma_start(out=w12z_sb[:], in_=w12z_d[:])

            wh4 = wh_sb[:].rearrange("p (i k m) -> p i k m", i=2, k=8)
            wm4 = wm_sb[:].rearrange("p (i s m) -> p i s m", i=2, s=4)
            w2q = w2h_sb[:].rearrange("p (q h) -> p q h", q=2)
            w12z4 = w12z_sb[:].rearrange("p (i c h) -> p i c h", i=2, c=4)

            acc_ps = ps_misc.tile([P, ML], fp32, tag="acc")

            n_macros = Mpad // 512
            seg_state = {"n": 0}

            def tail_a(ctx):
                """gate2(+glin fold) -> exp -> mlp2 -> y for macro ctx."""
                habs, f1_sb, m = ctx["habs"], ctx["f1"], ctx["m"]
                xq3, mo = ctx["xq3"], ctx["m"] * P
                g_ps_t = ps_misc.tile([P, 16], fp32, tag="gps")
                g_ps = g_ps_t[:]
                for c in range(4):
                    gc = g_ps[:, c * 4:(c + 1) * 4]
                    for q in range(2):
                        nc.tensor.matmul(
                            out=gc,
                            lhsT=habs[:, q * 512 + c * P:q * 512 + (c + 1) * P],
                            rhs=w2q[:, q, :],
                            start=(q == 0), stop=False)
                    # + linear gate part: c_lin * (W2 W1) x, from resident x
                    for i in range(2):
                        nc.tensor.matmul(
                            out=gc, lhsT=xq3[:, i, mo:mo + P],
                            rhs=w12z4[:, i, c, :],
                            start=False, stop=(i == 1))
                e_sb = epool.tile([P, 16], fp32, tag="esb")
                nc.scalar.activation(e_sb[:], g_ps, AF.Exp)
                f_ps = ps_f.tile([P, 512], fp32, tag="fps")
                for c in range(4):
                    nc.tensor.matmul(
                        out=f_ps[:, c * P:(c + 1) * P],
                        lhsT=f1_sb[:, c * P:(c + 1) * P],
                        rhs=m2_sb[:], start=True, stop=True)
                y_sb = ypool.tile([P, 2 * 2 * ML], fp8, tag="ysb")
                y4 = y_sb[:].rearrange("p (j i f) -> p j i f", j=2, i=2)
                nc.vector.tensor_tensor(
                    out=y4[:, :, :, 0:P].rearrange(
                        "p j i (h o) -> p j i h o", h=HEADS),
                    in0=f_ps[:].rearrange("p (j i h o) -> p j i h o",
                                          j=2, i=2, h=HEADS),
                    in1=e_sb[:].rearrange("p (j i h) -> p j i h", j=2, i=2)
                        .unsqueeze(4).broadcast_to([P, 2, 2, HEADS, OUT_CH]),
                    op=ALU.mult)
                nc.gpsimd.tensor_copy(
                    out=y4[:, :, :, P:ML],
                    in_=e_sb[:].rearrange("p (j i h) -> p j i h", j=2, i=2))
                ctx["y4"] = y4

            def tail_b(ctx):
                """segment accumulate for macro ctx (2 behind)."""
                ind5, m, y4 = ctx["ind5"], ctx["m"], ctx["y4"]
                for j in range(2):
                    nc.tensor.matmul(
                        out=acc_ps[:],
                        lhsT=ind5[:, m, j, :, :],
                        rhs=y4[:, j, :, :],
                        start=(seg_state["n"] == 0),
                        stop=(seg_state["n"] == 2 * n_macros - 1),
                        perf_mode=DR)
                    seg_state["n"] += 1

            pend = []
            boff = 0
            for bn in blocks:
                strip_b = bn // 4
                macros_b = bn // 512
                xq = xpool.tile([P, 2 * strip_b], fp8, tag="xq")
                nc.sync.dma_start(
                    out=xq[:], in_=xq_d[:, boff // 2:boff // 2 + 2 * strip_b])
                ind = xpool.tile([P, bn], fp8, tag="ind")
                nc.sync.dma_start(out=ind[:], in_=ind_d[:, boff:boff + bn])
                boff += bn
                xq3 = xq[:].rearrange("p (i n) -> p i n", i=2)
                ind5 = ind[:].rearrange("p (m j i s) -> p m j i s",
                                        m=macros_b, j=2, i=2)

                for m in range(macros_b):
                    mo = m * P
                    rhs = xq3[:, :, mo:mo + P]
                    # ---- gate1 + mlp1 (fp8 DoubleRow) ----
                    h_ps = ps_h.tile([P, 1024], fp32, tag="hps")
                    h4 = h_ps[:].rearrange("p (q s n) -> p q s n", q=2, s=4)
                    f1_ps = ps_f1.tile([P, 512], fp32, tag="f1ps")
                    f13 = f1_ps[:].rearrange("p (s n) -> p s n", s=4)
                    for s in range(4):
                        for q in range(2):
                            nc.tensor.matmul(
                                out=h4[:, q, s, :], lhsT=wh4[:, :, s * 2 + q, :],
                                rhs=rhs, start=True, stop=True, perf_mode=DR)
                        nc.tensor.matmul(
                            out=f13[:, s, :], lhsT=wm4[:, :, s, :],
                            rhs=rhs, start=True, stop=True, perf_mode=DR)

                    # ---- habs evac split across ACT / DVE / Pool ----
                    # (GPSIMD cannot read PSUM on real hw: evacs are ACT/DVE)
                    habs = hpool.tile([P, 1024], bf16, tag="habs")
                    c0 = HABS_ACT
                    nc.scalar.activation(habs[:, 0:c0], h_ps[:, 0:c0], AF.Abs)
                    if c0 < 1024:
                        nc.vector.tensor_scalar(
                            out=habs[:, c0:1024], in0=h_ps[:, c0:1024],
                            scalar1=0.0, scalar2=None, op0=ALU.abs_max)

                    # ---- f1 evac (bias + relu) on DVE ----
                    f1_sb = fpool.tile([P, 512], bf16, tag="f1sb")
                    nc.vector.tensor_scalar(
                        out=f1_sb[:], in0=f1_ps[:],
                        scalar1=b1_sb[:, 0:1], scalar2=0.0,
                        op0=ALU.add, op1=ALU.max)

                    # ---- software-pipelined tails: a(m-1), b(m-2) ----
                    pend.append({"habs": habs, "f1": f1_sb, "xq3": xq3,
                                 "ind5": ind5, "m": m})
                    if len(pend) >= 2:
                        tail_a(pend[-2])
                    if len(pend) >= 3:
                        tail_b(pend[0])
                        pend.pop(0)

            # drain pipeline
            tail_a(pend[-1])
            for ctx in pend:
                tail_b(ctx)

            # ---- final: out = num/den + b2 ----
            den_sb = cpool.tile([P, HEADS], fp32)
            nc.vector.tensor_scalar(
                out=den_sb[:], in0=acc_ps[:, P:ML], scalar1=1e-16,
                scalar2=None, op0=ALU.add)
            rec_sb = cpool.tile([P, HEADS], fp32)
            nc.vector.reciprocal(rec_sb[:], den_sb[:])
            out_sb = cpool.tile([P, P], fp32)
            nc.vector.tensor_tensor(
                out=out_sb[:].rearrange("p (h o) -> p h o", h=HEADS),
                in0=acc_ps[:, 0:P].rearrange("p (h o) -> p h o", h=HEADS),
                in1=rec_sb[:].unsqueeze(2).broadcast_to([P, HEADS, OUT_CH]),
                op=ALU.mult)
            nc.vector.tensor_tensor(
                out=out_sb[:], in0=out_sb[:], in1=b2_sb[:], op=ALU.add)
            nc.sync.dma_start(out=out_d[:], in_=out_sb[:])

    nc.compile()
    return nc


def _host_inputs(x, batch, gate_w1, prelu_a, gate_w2, mlp_w1, mlp_b1,
                 mlp_w2, mlp_b2, bnds, Mpad):
    """Build shared weight arrays + per-core input maps."""
    import ml_dtypes
    f8 = ml_dtypes.float8_e4m3
    bf = ml_dtypes.bfloat16

    a = float(np.asarray(prelu_a))
    c_abs = (1.0 - a) / 2.0
    c_lin = (1.0 + a) / 2.0

    # fp8 weights scaled x16 (away from subnormals); unscaled exactly via the
    # bf16 second-layer weights (w2h, m2t) and b1.
    WS = 16.0
    # wh[p, i, s*2+q, m] = [strip(p,i)==s] * WS*gate_w1[q*128+m, p%64]
    wh = np.zeros((P, 2, 8, P), np.float32)
    wm = np.zeros((P, 2, 4, P), np.float32)
    g1 = np.asarray(gate_w1, np.float32)      # [256, 64]
    m1 = np.asarray(mlp_w1, np.float32)       # [128, 64]
    for s in range(4):
        ph, i = s % 2, s // 2
        for q in range(2):
            wh[64 * ph:64 * (ph + 1), i, s * 2 + q, :] = \
                WS * g1[q * P:(q + 1) * P, :].T
        wm[64 * ph:64 * (ph + 1), i, s, :] = WS * m1.T
    w2h = (c_abs / WS * np.asarray(gate_w2, np.float32).T)  # [256, 4]
    w2h = w2h.reshape(2, P, HEADS).transpose(1, 0, 2)  # [128, 2, 4]
    # w12z[p, i, c, h] = [strip(p,i)==c] * c_lin * (gate_w2 @ gate_w1).T[ch, h]
    w12 = c_lin * (np.asarray(gate_w2, np.float32) @ g1).T  # [64, 4]
    w12z = np.zeros((P, 2, 4, HEADS), np.float32)
    for c in range(4):
        ph, i = c % 2, c // 2
        w12z[64 * ph:64 * (ph + 1), i, c, :] = w12

    shared = {
        "wh": np.ascontiguousarray(wh.reshape(P, -1)).astype(f8),
        "wm": np.ascontiguousarray(wm.reshape(P, -1)).astype(f8),
        "w2h": np.ascontiguousarray(w2h.reshape(P, -1)).astype(bf),
        "m2t": np.ascontiguousarray(
            np.asarray(mlp_w2, np.float32).T / WS).astype(bf),
        "b1": np.ascontiguousarray(
            WS * np.asarray(mlp_b1, np.float32).reshape(P, 1)),
        "b2r": np.ascontiguousarray(
            np.tile(np.asarray(mlp_b2, np.float32).reshape(1, P), (P, 1))),
        "w12z": np.ascontiguousarray(w12z.reshape(P, -1)).astype(bf),
    }

    blocks = [BLK_NODES] * (Mpad // BLK_NODES)
    if Mpad % BLK_NODES:
        blocks.append(Mpad % BLK_NODES)
    in_maps = []
    for c in range(len(bnds) - 1):
        r0, r1 = int(bnds[c]), int(bnds[c + 1])
        cnt = r1 - r0
        xs = np.zeros((Mpad, IN_CH), np.float32)
        xs[:cnt] = x[r0:r1]
        x8 = xs.astype(f8)
        onehot = np.zeros((Mpad, P), np.uint8)
        bid = (batch[r0:r1] - c * SEGS_PER_CORE).astype(np.int64)
        onehot[np.arange(cnt), bid] = 1
        # per block: xq[p, i*sb + col] = x[node(b, 2i+p//64, col), p%64]
        #            ind[p, (m, c, s)] = onehot[node(b, c, m*128+p), s]
        xq = np.empty((P, Mpad // 2), f8)
        ind = np.empty((P, Mpad), np.uint8)
        noff = 0
        for bn in blocks:
            sb = bn // 4
            xb = x8[noff:noff + bn].reshape(4, sb, IN_CH)
            for i in range(2):
                for ph in range(2):
                    xq[64 * ph:64 * (ph + 1),
                       noff // 2 + i * sb:noff // 2 + (i + 1) * sb] = \
                        xb[2 * i + ph].T
            ib = onehot[noff:noff + bn].reshape(4, bn // 512, P, P)
            ind[:, noff:noff + bn] = \
                ib.transpose(2, 1, 0, 3).reshape(P, bn)
            noff += bn
        in_maps.append({
            "xq": np.ascontiguousarray(xq),
            "ind": np.ascontiguousarray(ind).astype(f8),
            **shared,
        })
    return in_maps


def kernel(x, batch, num_segments, gate_w1, prelu_a, gate_w2,
           mlp_w1, mlp_b1, mlp_w2, mlp_b2):
    from concourse.bass_utils import run_bass_kernel_spmd

    x = np.asarray(x, dtype=np.float32)
    batch = np.asarray(batch, dtype=np.int32)

    bnds = np.searchsorted(batch, np.arange(0, NUM_SEGS + 1, SEGS_PER_CORE))
    counts = np.diff(bnds)
    Mpad = int(-(-counts.max() // 2048) * 2048)

    nc = _build_bass(Mpad)
    in_maps = _host_inputs(x, batch, gate_w1, prelu_a, gate_w2, mlp_w1,
                           mlp_b1, mlp_w2, mlp_b2, bnds, Mpad)
    res = run_bass_kernel_spmd(nc, in_maps, core_ids=list(range(N_CORES)))
    out = np.concatenate([res.results[c]["out"] for c in range(N_CORES)],
                         axis=0)
    return out.astype(np.float32)


# revision 7
# speedup vs baseline: 1.0836x; 1.0082x over previous
"""MultiHeadGlobalAttention (segment softmax attention pooling) on 8 trn2 cores, v2.

Sharding: 128 segments/core (batch sorted -> contiguous node ranges), weights
replicated. Per-core blocks of 4096 nodes = 4 strips x 1024; macro = 512 nodes
(128 node-columns x 4 strips packed into the fp8 DoubleRow contraction grid of
128 partitions x 2 pairs).

Device pipeline per macro (software-pipelined: front(m) | gate2..y(m-1) |
segment-reduce(m-2)):
  gate1(h, 8 mm) + mlp1(f1, 4 mm) as fp8e4 DoubleRow (block-"diagonal" strip
  weights, x16-scaled weights unscaled via the bf16 second-layer weights);
  |h| evac on ACT -> bf16; f1 evac (bias+relu) on DVE -> bf16; gate2 as plain
  bf16 accumulating matmuls with the host linear-gate part (c_lin*W2W1)
  folded in as two extra matmuls against the resident fp8 x; exp on ACT;
  mlp2 plain bf16 data-stationary; y = e*f2 on DVE -> fp8; e concat on Pool
  (SBUF-only: GPSIMD cannot access PSUM); segment-reduce via host-DMA'd fp8
  indicator in DoubleRow chunk-pairs into resident PSUM acc [128 segs, 132];
  final divide + output bias on DVE. Blocks of 4096 nodes (+ optional 2048
  tail block).
"""

import sys

for _p in ("/opt/trn_rl_repo", "/root/.axon_site/_ro/trn_rl_repo"):
    if _p not in sys.path:
        sys.path.append(_p)

import numpy as np

IN_CH = 64
OUT_CH = 32
HEADS = 4
NUM_SEGS = 1024
N_CORES = 8
SEGS_PER_CORE = NUM_SEGS // N_CORES  # 128

P = 128
BLK_NODES = 4096
STRIP = 1024            # nodes per strip per block (4 strips)
MACROS_PER_BLK = 8      # each macro: 128 node-cols x 4 strips = 512 nodes
ML = P + HEADS          # 132: 128 feat cols + 4 e cols

# habs evac split point (flat cols of [128, 1024]): [0:HABS_ACT) on ACT,
# rest on DVE. PSUM evacuation is legal only on ACT/DVE (GPSIMD cannot
# read PSUM on real hardware).
HABS_ACT = 1024


def _build_bass(Mpad):
    import concourse.bacc as bacc
    import concourse.tile as tile
    from concourse import mybir

    fp32 = mybir.dt.float32
    bf16 = mybir.dt.bfloat16
    fp8 = mybir.dt.float8e4
    AF = mybir.ActivationFunctionType
    ALU = mybir.AluOpType
    DR = mybir.MatmulPerfMode.DoubleRow

    nc = bacc.Bacc("TRN2", target_bir_lowering=False, debug=False)

    xq_d = nc.dram_tensor("xq", [P, Mpad // 2], fp8, kind="ExternalInput")
    ind_d = nc.dram_tensor("ind", [P, Mpad], fp8, kind="ExternalInput")
    w12z_d = nc.dram_tensor("w12z", [P, 2 * 4 * HEADS], bf16,
                            kind="ExternalInput")
    wh_d = nc.dram_tensor("wh", [P, 2 * 8 * P], fp8, kind="ExternalInput")
    wm_d = nc.dram_tensor("wm", [P, 2 * 4 * P], fp8, kind="ExternalInput")
    w2h_d = nc.dram_tensor("w2h", [P, 2 * HEADS], bf16, kind="ExternalInput")
    m2_d = nc.dram_tensor("m2t", [P, P], bf16, kind="ExternalInput")
    b1_d = nc.dram_tensor("b1", [P, 1], fp32, kind="ExternalInput")
    b2_d = nc.dram_tensor("b2r", [P, P], fp32, kind="ExternalInput")
    out_d = nc.dram_tensor("out", [P, P], fp32, kind="ExternalOutput")

    assert Mpad % 2048 == 0
    blocks = [BLK_NODES] * (Mpad // BLK_NODES)
    if Mpad % BLK_NODES:
        blocks.append(Mpad % BLK_NODES)

    with tile.TileContext(nc) as tc:
        with (
            tc.tile_pool(name="const", bufs=1) as cpool,
            tc.tile_pool(name="xin", bufs=2) as xpool,
            tc.tile_pool(name="hsb", bufs=4) as hpool,
            tc.tile_pool(name="fsb", bufs=4) as fpool,
            tc.tile_pool(name="ysb", bufs=4) as ypool,
            tc.tile_pool(name="esb", bufs=4) as epool,
            tc.tile_pool(name="ps_h", bufs=2, space="PSUM") as ps_h,
            tc.tile_pool(name="ps_f1", bufs=1, space="PSUM") as ps_f1,
            tc.tile_pool(name="ps_f", bufs=1, space="PSUM") as ps_f,
            tc.tile_pool(name="ps_misc", bufs=1, space="PSUM") as ps_misc,
        ):
            # ---- static setup ----
            # weights ride the ACT hwdge queue (idle during the prologue) so
            # they overlap the first block's xq/ind DMAs on SP; b1 + a warm
            # activation go first so the act-table load (~1.3us) overlaps too.
            b1_sb = cpool.tile([P, 1], fp32)
            nc.scalar.dma_start(out=b1_sb[:], in_=b1_d[:])
            warm_sb = cpool.tile([P, 1], fp32)
            nc.scalar.activation(warm_sb[:], b1_sb[:], AF.Abs)
            wh_sb = cpool.tile([P, 2 * 8 * P], fp8)
            nc.scalar.dma_start(out=wh_sb[:], in_=wh_d[:])
            wm_sb = cpool.tile([P, 2 * 4 * P], fp8)
            nc.scalar.dma_start(out=wm_sb[:], in_=wm_d[:])
            w2h_sb = cpool.tile([P, 2 * HEADS], bf16)
            nc.scalar.dma_start(out=w2h_sb[:], in_=w2h_d[:])
            m2_sb = cpool.tile([P, P], bf16)
            nc.scalar.dma_start(out=m2_sb[:], in_=m2_d[:])
            b2_sb = cpool.tile([P, P], fp32)
            nc.scalar.dma_start(out=b2_sb[:], in_=b2_d[:])
            w12z_sb = cpool.tile([P, 2 * 4 * HEADS], bf16)
            nc.scalar.dma_start(out=w12z_sb[:], in_=w12z_d[:])

            wh4 = wh_sb[:].rearrange("p (i k m) -> p i k m", i=2, k=8)
            wm4 = wm_sb[:].rearrange("p (i s m) -> p i s m", i=2, s=4)
            w2q = w2h_sb[:].rearrange("p (q h) -> p q h", q=2)
            w12z4 = w12z_sb[:].rearrange("p (i c h) -> p i c h", i=2, c=4)

            acc_ps = ps_misc.tile([P, ML], fp32, tag="acc")

            n_macros = Mpad // 512
            seg_state = {"n": 0}

            def tail_a_head(ctx):
                """gate2(+glin fold) -> exp for macro ctx (1 behind)."""
                habs, m = ctx["habs"], ctx["m"]
                xq3, mo = ctx["xq3"], ctx["m"] * P
                g_ps_t = ps_misc.tile([P, 16], fp32, tag="gps")
                g_ps = g_ps_t[:]
                for c in range(4):
                    gc = g_ps[:, c * 4:(c + 1) * 4]
                    for q in range(2):
                        nc.tensor.matmul(
                            out=gc,
                            lhsT=habs[:, q * 512 + c * P:q * 512 + (c + 1) * P],
                            rhs=w2q[:, q, :],
                            start=(q == 0), stop=False)
                    # + linear gate part: c_lin * (W2 W1) x, from resident x
                    for i in range(2):
                        nc.tensor.matmul(
                            out=gc, lhsT=xq3[:, i, mo:mo + P],
                            rhs=w12z4[:, i, c, :],
                            start=False, stop=(i == 1))
                e_sb = epool.tile([P, 16], fp32, tag="esb")
                nc.scalar.activation(e_sb[:], g_ps, AF.Exp)
                ctx["e_sb"] = e_sb

            def tail_a(ctx):
                """mlp2 -> y for macro ctx (1 behind)."""
                f1_sb, e_sb = ctx["f1"], ctx["e_sb"]
                f_ps = ps_f.tile([P, 512], fp32, tag="fps")
                for c in range(4):
                    nc.tensor.matmul(
                        out=f_ps[:, c * P:(c + 1) * P],
                        lhsT=f1_sb[:, c * P:(c + 1) * P],
                        rhs=m2_sb[:], start=True, stop=True)
                y_sb = ypool.tile([P, 2 * 2 * ML], fp8, tag="ysb")
                y4 = y_sb[:].rearrange("p (j i f) -> p j i f", j=2, i=2)
                nc.vector.tensor_tensor(
                    out=y4[:, :, :, 0:P].rearrange(
                        "p j i (h o) -> p j i h o", h=HEADS),
                    in0=f_ps[:].rearrange("p (j i h o) -> p j i h o",
                                          j=2, i=2, h=HEADS),
                    in1=e_sb[:].rearrange("p (j i h) -> p j i h", j=2, i=2)
                        .unsqueeze(4).broadcast_to([P, 2, 2, HEADS, OUT_CH]),
                    op=ALU.mult)
                nc.gpsimd.tensor_copy(
                    out=y4[:, :, :, P:ML],
                    in_=e_sb[:].rearrange("p (j i h) -> p j i h", j=2, i=2))
                ctx["y4"] = y4

            def tail_b(ctx):
                """segment accumulate for macro ctx (2 behind)."""
                ind5, m, y4 = ctx["ind5"], ctx["m"], ctx["y4"]
                for j in range(2):
                    nc.tensor.matmul(
                        out=acc_ps[:],
                        lhsT=ind5[:, m, j, :, :],
                        rhs=y4[:, j, :, :],
                        start=(seg_state["n"] == 0),
                        stop=(seg_state["n"] == 2 * n_macros - 1),
                        perf_mode=DR)
                    seg_state["n"] += 1

            pend = []
            boff = 0
            for bn in blocks:
                strip_b = bn // 4
                macros_b = bn // 512
                xq = xpool.tile([P, 2 * strip_b], fp8, tag="xq")
                nc.sync.dma_start(
                    out=xq[:], in_=xq_d[:, boff // 2:boff // 2 + 2 * strip_b])
                ind = xpool.tile([P, bn], fp8, tag="ind")
                nc.sync.dma_start(out=ind[:], in_=ind_d[:, boff:boff + bn])
                boff += bn
                xq3 = xq[:].rearrange("p (i n) -> p i n", i=2)
                ind5 = ind[:].rearrange("p (m j i s) -> p m j i s",
                                        m=macros_b, j=2, i=2)

                for m in range(macros_b):
                    mo = m * P
                    rhs = xq3[:, :, mo:mo + P]
                    # ---- gate1 + mlp1 (fp8 DoubleRow) ----
                    h_ps = ps_h.tile([P, 1024], fp32, tag="hps")
                    h4 = h_ps[:].rearrange("p (q s n) -> p q s n", q=2, s=4)
                    f1_ps = ps_f1.tile([P, 512], fp32, tag="f1ps")
                    f13 = f1_ps[:].rearrange("p (s n) -> p s n", s=4)
                    for s in range(4):
                        for q in range(2):
                            nc.tensor.matmul(
                                out=h4[:, q, s, :], lhsT=wh4[:, :, s * 2 + q, :],
                                rhs=rhs, start=True, stop=True, perf_mode=DR)
                        nc.tensor.matmul(
                            out=f13[:, s, :], lhsT=wm4[:, :, s, :],
                            rhs=rhs, start=True, stop=True, perf_mode=DR)

                    if len(pend) >= 1:
                        tail_a_head(pend[-1])
                    # ---- habs evac split across ACT / DVE / Pool ----
                    # (GPSIMD cannot read PSUM on real hw: evacs are ACT/DVE)
                    habs = hpool.tile([P, 1024], bf16, tag="habs")
                    c0 = HABS_ACT
                    nc.scalar.activation(habs[:, 0:c0], h_ps[:, 0:c0], AF.Abs)
                    if c0 < 1024:
                        nc.vector.tensor_scalar(
                            out=habs[:, c0:1024], in0=h_ps[:, c0:1024],
                            scalar1=0.0, scalar2=None, op0=ALU.abs_max)

                    # ---- f1 evac (bias + relu) on DVE ----
                    f1_sb = fpool.tile([P, 512], bf16, tag="f1sb")
                    nc.vector.tensor_scalar(
                        out=f1_sb[:], in0=f1_ps[:],
                        scalar1=b1_sb[:, 0:1], scalar2=0.0,
                        op0=ALU.add, op1=ALU.max)

                    # ---- software-pipelined tails: a(m-1), b(m-2) ----
                    pend.append({"habs": habs, "f1": f1_sb, "xq3": xq3,
                                 "ind5": ind5, "m": m})
                    if len(pend) >= 2:
                        tail_a(pend[-2])
                    if len(pend) >= 3:
                        tail_b(pend[0])
                        pend.pop(0)

            # drain pipeline
            tail_a_head(pend[-1])
            tail_a(pend[-1])
            for ctx in pend:
                tail_b(ctx)

            # ---- final: out = num/den + b2 ----
            den_sb = cpool.tile([P, HEADS], fp32)
            nc.vector.tensor_scalar(
                out=den_sb[:], in0=acc_ps[:, P:ML], scalar1=1e-16,
                scalar2=None, op0=ALU.add)
            rec_sb = cpool.tile([P, HEADS], fp32)
            nc.vector.reciprocal(rec_sb[:], den_sb[:])
            out_sb = cpool.tile([P, P], fp32)
            nc.vector.tensor_tensor(
                out=out_sb[:].rearrange("p (h o) -> p h o", h=HEADS),
                in0=acc_ps[:, 0:P].rearrange("p (h o) -> p h o", h=HEADS),
                in1=rec_sb[:].unsqueeze(2).broadcast_to([P, HEADS, OUT_CH]),
                op=ALU.mult)
            nc.vector.tensor_tensor(
                out=out_sb[:], in0=out_sb[:], in1=b2_sb[:], op=ALU.add)
            nc.sync.dma_start(out=out_d[:], in_=out_sb[:])

    nc.compile()
    return nc


def _host_inputs(x, batch, gate_w1, prelu_a, gate_w2, mlp_w1, mlp_b1,
                 mlp_w2, mlp_b2, bnds, Mpad):
    """Build shared weight arrays + per-core input maps."""
    import ml_dtypes
    f8 = ml_dtypes.float8_e4m3
    bf = ml_dtypes.bfloat16

    a = float(np.asarray(prelu_a))
    c_abs = (1.0 - a) / 2.0
    c_lin = (1.0 + a) / 2.0

    # fp8 weights scaled x16 (away from subnormals); unscaled exactly via the
    # bf16 second-layer weights (w2h, m2t) and b1.
    WS = 16.0
    # wh[p, i, s*2+q, m] = [strip(p,i)==s] * WS*gate_w1[q*128+m, p%64]
    wh = np.zeros((P, 2, 8, P), np.float32)
    wm = np.zeros((P, 2, 4, P), np.float32)
    g1 = np.asarray(gate_w1, np.float32)      # [256, 64]
    m1 = np.asarray(mlp_w1, np.float32)       # [128, 64]
    for s in range(4):
        ph, i = s % 2, s // 2
        for q in range(2):
            wh[64 * ph:64 * (ph + 1), i, s * 2 + q, :] = \
                WS * g1[q * P:(q + 1) * P, :].T
        wm[64 * ph:64 * (ph + 1), i, s, :] = WS * m1.T
    w2h = (c_abs / WS * np.asarray(gate_w2, np.float32).T)  # [256, 4]
    w2h = w2h.reshape(2, P, HEADS).transpose(1, 0, 2)  # [128, 2, 4]
    # w12z[p, i, c, h] = [strip(p,i)==c] * c_lin * (gate_w2 @ gate_w1).T[ch, h]
    w12 = c_lin * (np.asarray(gate_w2, np.float32) @ g1).T  # [64, 4]
    w12z = np.zeros((P, 2, 4, HEADS), np.float32)
    for c in range(4):
        ph, i = c % 2, c // 2
        w12z[64 * ph:64 * (ph + 1), i, c, :] = w12

    shared = {
        "wh": np.ascontiguousarray(wh.reshape(P, -1)).astype(f8),
        "wm": np.ascontiguousarray(wm.reshape(P, -1)).astype(f8),
        "w2h": np.ascontiguousarray(w2h.reshape(P, -1)).astype(bf),
        "m2t": np.ascontiguousarray(
            np.asarray(mlp_w2, np.float32).T / WS).astype(bf),
        "b1": np.ascontiguousarray(
            WS * np.asarray(mlp_b1, np.float32).reshape(P, 1)),
        "b2r": np.ascontiguousarray(
            np.tile(np.asarray(mlp_b2, np.float32).reshape(1, P), (P, 1))),
        "w12z": np.ascontiguousarray(w12z.reshape(P, -1)).astype(bf),
    }

    blocks = [BLK_NODES] * (Mpad // BLK_NODES)
    if Mpad % BLK_NODES:
        blocks.append(Mpad % BLK_NODES)
    in_maps = []
    for c in range(len(bnds) - 1):
        r0, r1 = int(bnds[c]), int(bnds[c + 1])
        cnt = r1 - r0
        xs = np.zeros((Mpad, IN_CH), np.float32)
        xs[:cnt] = x[r0:r1]
        x8 = xs.astype(f8)
        onehot = np.zeros((Mpad, P), np.uint8)
        bid = (batch[r0:r1] - c * SEGS_PER_CORE).astype(np.int64)
        onehot[np.arange(cnt), bid] = 1
        # per block: xq[p, i*sb + col] = x[node(b, 2i+p//64, col), p%64]
        #            ind[p, (m, c, s)] = onehot[node(b, c, m*128+p), s]
        xq = np.empty((P, Mpad // 2), f8)
        ind = np.empty((P, Mpad), np.uint8)
        noff = 0
        for bn in blocks:
            sb = bn // 4
            xb = x8[noff:noff + bn].reshape(4, sb, IN_CH)
            for i in range(2):
                for ph in range(2):
                    xq[64 * ph:64 * (ph + 1),
                       noff // 2 + i * sb:noff // 2 + (i + 1) * sb] = \
                        xb[2 * i + ph].T
            ib = onehot[noff:noff + bn].reshape(4, bn // 512, P, P)
            ind[:, noff:noff + bn] = \
                ib.transpose(2, 1, 0, 3).reshape(P, bn)
            noff += bn
        in_maps.append({
            "xq": np.ascontiguousarray(xq),
            "ind": np.ascontiguousarray(ind).astype(f8),
            **shared,
        })
    return in_maps


def kernel(x, batch, num_segments, gate_w1, prelu_a, gate_w2,
           mlp_w1, mlp_b1, mlp_w2, mlp_b2):
    from concourse.bass_utils import run_bass_kernel_spmd

    x = np.asarray(x, dtype=np.float32)
    batch = np.asarray(batch, dtype=np.int32)

    bnds = np.searchsorted(batch, np.arange(0, NUM_SEGS + 1, SEGS_PER_CORE))
    counts = np.diff(bnds)
    Mpad = int(-(-counts.max() // 2048) * 2048)

    nc = _build_bass(Mpad)
    in_maps = _host_inputs(x, batch, gate_w1, prelu_a, gate_w2, mlp_w1,
                           mlp_b1, mlp_w2, mlp_b2, bnds, Mpad)
    res = run_bass_kernel_spmd(nc, in_maps, core_ids=list(range(N_CORES)))
    out = np.concatenate([res.results[c]["out"] for c in range(N_CORES)],
                         axis=0)
    return out.astype(np.float32)


# revision 8
# speedup vs baseline: 1.0986x; 1.0139x over previous
"""MultiHeadGlobalAttention (segment softmax attention pooling) on 8 trn2 cores, v2.

Sharding: 128 segments/core (batch sorted -> contiguous node ranges), weights
replicated. Per-core blocks of 4096 nodes = 4 strips x 1024; macro = 512 nodes
(128 node-columns x 4 strips packed into the fp8 DoubleRow contraction grid of
128 partitions x 2 pairs).

Device pipeline per macro (software-pipelined: front(m) | gate2..y(m-1) |
segment-reduce(m-2)):
  gate1(h, 8 mm) + mlp1(f1, 4 mm) as fp8e4 DoubleRow (block-"diagonal" strip
  weights, x16-scaled weights unscaled via the bf16 second-layer weights);
  |h| evac on ACT -> bf16; f1 evac (bias+relu) on DVE -> bf16; gate2 as plain
  bf16 accumulating matmuls with the host linear-gate part (c_lin*W2W1)
  folded in as two extra matmuls against the resident fp8 x; exp on ACT;
  mlp2 plain bf16 data-stationary; y = e*f2 on DVE -> fp8; e concat on Pool
  (SBUF-only: GPSIMD cannot access PSUM); segment-reduce via host-DMA'd fp8
  indicator in DoubleRow chunk-pairs into resident PSUM acc [128 segs, 132];
  final divide + output bias on DVE. Blocks of 4096 nodes (+ optional 2048
  tail block).
"""

import sys

for _p in ("/opt/trn_rl_repo", "/root/.axon_site/_ro/trn_rl_repo"):
    if _p not in sys.path:
        sys.path.append(_p)

import numpy as np

IN_CH = 64
OUT_CH = 32
HEADS = 4
NUM_SEGS = 1024
N_CORES = 8
SEGS_PER_CORE = NUM_SEGS // N_CORES  # 128

P = 128
BLK_NODES = 4096
STRIP = 1024            # nodes per strip per block (4 strips)
MACROS_PER_BLK = 8      # each macro: 128 node-cols x 4 strips = 512 nodes
ML = P + HEADS          # 132: 128 feat cols + 4 e cols

# habs evac split point (flat cols of [128, 1024]): [0:HABS_ACT) on ACT,
# rest on DVE. PSUM evacuation is legal only on ACT/DVE (GPSIMD cannot
# read PSUM on real hardware).
HABS_ACT = 1024


def _build_bass(Mpad):
    import concourse.bacc as bacc
    import concourse.tile as tile
    from concourse import mybir

    fp32 = mybir.dt.float32
    bf16 = mybir.dt.bfloat16
    fp8 = mybir.dt.float8e4
    AF = mybir.ActivationFunctionType
    ALU = mybir.AluOpType
    DR = mybir.MatmulPerfMode.DoubleRow

    nc = bacc.Bacc("TRN2", target_bir_lowering=False, debug=False)

    xq_d = nc.dram_tensor("xq", [P, Mpad // 2], fp8, kind="ExternalInput")
    ind_d = nc.dram_tensor("ind", [P, Mpad], fp8, kind="ExternalInput")
    w12z_d = nc.dram_tensor("w12z", [P, 2 * 4 * HEADS], bf16,
                            kind="ExternalInput")
    wh_d = nc.dram_tensor("wh", [P, 2 * 8 * P], fp8, kind="ExternalInput")
    wm_d = nc.dram_tensor("wm", [P, 2 * 4 * P], fp8, kind="ExternalInput")
    w2h_d = nc.dram_tensor("w2h", [P, 2 * HEADS], bf16, kind="ExternalInput")
    m2_d = nc.dram_tensor("m2t", [P, P], bf16, kind="ExternalInput")
    b1_d = nc.dram_tensor("b1", [P, 1], fp32, kind="ExternalInput")
    b2_d = nc.dram_tensor("b2r", [P, P], fp32, kind="ExternalInput")
    out_d = nc.dram_tensor("out", [P, P], fp32, kind="ExternalOutput")

    assert Mpad % 2048 == 0
    blocks = [BLK_NODES] * (Mpad // BLK_NODES)
    if Mpad % BLK_NODES:
        blocks.append(Mpad % BLK_NODES)

    with tile.TileContext(nc) as tc:
        with (
            tc.tile_pool(name="const", bufs=1) as cpool,
            tc.tile_pool(name="xin", bufs=2) as xpool,
            tc.tile_pool(name="hsb", bufs=4) as hpool,
            tc.tile_pool(name="fsb", bufs=4) as fpool,
            tc.tile_pool(name="ysb", bufs=4) as ypool,
            tc.tile_pool(name="esb", bufs=4) as epool,
            tc.tile_pool(name="ps_h", bufs=2, space="PSUM") as ps_h,
            tc.tile_pool(name="ps_f1", bufs=1, space="PSUM") as ps_f1,
            tc.tile_pool(name="ps_f", bufs=1, space="PSUM") as ps_f,
            tc.tile_pool(name="ps_misc", bufs=1, space="PSUM") as ps_misc,
        ):
            # ---- static setup ----
            # Matmul weights first on the ACT hwdge queue (idle in the
            # prologue) so PE can start ~1.5us in; then the act-table warm
            # load. Small consts ride the gpsimd SWDGE queue (also idle) so
            # the first habs isn't queued behind them on ACT. SP carries the
            # first block's xq/ind.
            b1_sb = cpool.tile([P, 1], fp32)
            nc.gpsimd.dma_start(out=b1_sb[:], in_=b1_d[:])
            wh_sb = cpool.tile([P, 2 * 8 * P], fp8)
            nc.gpsimd.dma_start(out=wh_sb[:], in_=wh_d[:])
            wm_sb = cpool.tile([P, 2 * 4 * P], fp8)
            nc.gpsimd.dma_start(out=wm_sb[:], in_=wm_d[:])
            warm_sb = cpool.tile([P, 1], fp32)
            nc.scalar.activation(warm_sb[:], b1_sb[:], AF.Abs)
            w2h_sb = cpool.tile([P, 2 * HEADS], bf16)
            nc.sync.dma_start(out=w2h_sb[:], in_=w2h_d[:])
            m2_sb = cpool.tile([P, P], bf16)
            nc.sync.dma_start(out=m2_sb[:], in_=m2_d[:])
            b2_sb = cpool.tile([P, P], fp32)
            nc.sync.dma_start(out=b2_sb[:], in_=b2_d[:])
            w12z_sb = cpool.tile([P, 2 * 4 * HEADS], bf16)
            nc.sync.dma_start(out=w12z_sb[:], in_=w12z_d[:])

            wh4 = wh_sb[:].rearrange("p (i k m) -> p i k m", i=2, k=8)
            wm4 = wm_sb[:].rearrange("p (i s m) -> p i s m", i=2, s=4)
            w2q = w2h_sb[:].rearrange("p (q h) -> p q h", q=2)
            w12z4 = w12z_sb[:].rearrange("p (i c h) -> p i c h", i=2, c=4)

            acc_ps = ps_misc.tile([P, ML], fp32, tag="acc")

            n_macros = Mpad // 512
            seg_state = {"n": 0}

            def tail_a_head(ctx):
                """gate2(+glin fold) -> exp for macro ctx (1 behind)."""
                habs, m = ctx["habs"], ctx["m"]
                xq3, mo = ctx["xq3"], ctx["m"] * P
                g_ps_t = ps_misc.tile([P, 16], fp32, tag="gps")
                g_ps = g_ps_t[:]
                for c in range(4):
                    gc = g_ps[:, c * 4:(c + 1) * 4]
                    for q in range(2):
                        nc.tensor.matmul(
                            out=gc,
                            lhsT=habs[:, q * 512 + c * P:q * 512 + (c + 1) * P],
                            rhs=w2q[:, q, :],
                            start=(q == 0), stop=False)
                    # + linear gate part: c_lin * (W2 W1) x, from resident x
                    for i in range(2):
                        nc.tensor.matmul(
                            out=gc, lhsT=xq3[:, i, mo:mo + P],
                            rhs=w12z4[:, i, c, :],
                            start=False, stop=(i == 1))
                e_sb = epool.tile([P, 16], fp32, tag="esb")
                nc.scalar.activation(e_sb[:], g_ps, AF.Exp)
                ctx["e_sb"] = e_sb

            def tail_a(ctx):
                """mlp2 -> y for macro ctx (1 behind)."""
                f1_sb, e_sb = ctx["f1"], ctx["e_sb"]
                f_ps = ps_f.tile([P, 512], fp32, tag="fps")
                for c in range(4):
                    nc.tensor.matmul(
                        out=f_ps[:, c * P:(c + 1) * P],
                        lhsT=f1_sb[:, c * P:(c + 1) * P],
                        rhs=m2_sb[:], start=True, stop=True)
                y_sb = ypool.tile([P, 2 * 2 * ML], fp8, tag="ysb")
                y4 = y_sb[:].rearrange("p (j i f) -> p j i f", j=2, i=2)
                nc.vector.tensor_tensor(
                    out=y4[:, :, :, 0:P].rearrange(
                        "p j i (h o) -> p j i h o", h=HEADS),
                    in0=f_ps[:].rearrange("p (j i h o) -> p j i h o",
                                          j=2, i=2, h=HEADS),
                    in1=e_sb[:].rearrange("p (j i h) -> p j i h", j=2, i=2)
                        .unsqueeze(4).broadcast_to([P, 2, 2, HEADS, OUT_CH]),
                    op=ALU.mult)
                nc.gpsimd.tensor_copy(
                    out=y4[:, :, :, P:ML],
                    in_=e_sb[:].rearrange("p (j i h) -> p j i h", j=2, i=2))
                ctx["y4"] = y4

            def tail_b(ctx):
                """segment accumulate for macro ctx (2 behind)."""
                ind5, m, y4 = ctx["ind5"], ctx["m"], ctx["y4"]
                for j in range(2):
                    nc.tensor.matmul(
                        out=acc_ps[:],
                        lhsT=ind5[:, m, j, :, :],
                        rhs=y4[:, j, :, :],
                        start=(seg_state["n"] == 0),
                        stop=(seg_state["n"] == 2 * n_macros - 1),
                        perf_mode=DR)
                    seg_state["n"] += 1

            pend = []
            boff = 0
            for bn in blocks:
                strip_b = bn // 4
                macros_b = bn // 512
                xq = xpool.tile([P, 2 * strip_b], fp8, tag="xq")
                nc.sync.dma_start(
                    out=xq[:], in_=xq_d[:, boff // 2:boff // 2 + 2 * strip_b])
                ind = xpool.tile([P, bn], fp8, tag="ind")
                nc.sync.dma_start(out=ind[:], in_=ind_d[:, boff:boff + bn])
                boff += bn
                xq3 = xq[:].rearrange("p (i n) -> p i n", i=2)
                ind5 = ind[:].rearrange("p (m j i s) -> p m j i s",
                                        m=macros_b, j=2, i=2)

                for m in range(macros_b):
                    mo = m * P
                    rhs = xq3[:, :, mo:mo + P]
                    # ---- gate1 + mlp1 (fp8 DoubleRow) ----
                    h_ps = ps_h.tile([P, 1024], fp32, tag="hps")
                    h4 = h_ps[:].rearrange("p (q s n) -> p q s n", q=2, s=4)
                    f1_ps = ps_f1.tile([P, 512], fp32, tag="f1ps")
                    f13 = f1_ps[:].rearrange("p (s n) -> p s n", s=4)
                    for s in range(4):
                        for q in range(2):
                            nc.tensor.matmul(
                                out=h4[:, q, s, :], lhsT=wh4[:, :, s * 2 + q, :],
                                rhs=rhs, start=True, stop=True, perf_mode=DR)
                        nc.tensor.matmul(
                            out=f13[:, s, :], lhsT=wm4[:, :, s, :],
                            rhs=rhs, start=True, stop=True, perf_mode=DR)

                    if len(pend) >= 1:
                        tail_a_head(pend[-1])
                    # ---- habs evac split across ACT / DVE / Pool ----
                    # (GPSIMD cannot read PSUM on real hw: evacs are ACT/DVE)
                    habs = hpool.tile([P, 1024], bf16, tag="habs")
                    c0 = HABS_ACT
                    nc.scalar.activation(habs[:, 0:c0], h_ps[:, 0:c0], AF.Abs)
                    if c0 < 1024:
                        nc.vector.tensor_scalar(
                            out=habs[:, c0:1024], in0=h_ps[:, c0:1024],
                            scalar1=0.0, scalar2=None, op0=ALU.abs_max)

                    # ---- f1 evac (bias + relu) on DVE ----
                    f1_sb = fpool.tile([P, 512], bf16, tag="f1sb")
                    nc.vector.tensor_scalar(
                        out=f1_sb[:], in0=f1_ps[:],
                        scalar1=b1_sb[:, 0:1], scalar2=0.0,
                        op0=ALU.add, op1=ALU.max)

                    # ---- software-pipelined tails: a(m-1), b(m-2) ----
                    pend.append({"habs": habs, "f1": f1_sb, "xq3": xq3,
                                 "ind5": ind5, "m": m})
                    if len(pend) >= 2:
                        tail_a(pend[-2])
                    if len(pend) >= 3:
                        tail_b(pend[0])
                        pend.pop(0)

            # drain pipeline
            tail_a_head(pend[-1])
            tail_a(pend[-1])
            for ctx in pend:
                tail_b(ctx)

            # ---- final: out = num/den + b2 ----
            den_sb = cpool.tile([P, HEADS], fp32)
            nc.vector.tensor_scalar(
                out=den_sb[:], in0=acc_ps[:, P:ML], scalar1=1e-16,
                scalar2=None, op0=ALU.add)
            rec_sb = cpool.tile([P, HEADS], fp32)
            nc.vector.reciprocal(rec_sb[:], den_sb[:])
            out_sb = cpool.tile([P, P], fp32)
            nc.vector.tensor_tensor(
                out=out_sb[:].rearrange("p (h o) -> p h o", h=HEADS),
                in0=acc_ps[:, 0:P].rearrange("p (h o) -> p h o", h=HEADS),
                in1=rec_sb[:].unsqueeze(2).broadcast_to([P, HEADS, OUT_CH]),
                op=ALU.mult)
            nc.vector.tensor_tensor(
                out=out_sb[:], in0=out_sb[:], in1=b2_sb[:], op=ALU.add)
            nc.sync.dma_start(out=out_d[:], in_=out_sb[:])

    nc.compile()
    return nc


def _host_inputs(x, batch, gate_w1, prelu_a, gate_w2, mlp_w1, mlp_b1,
                 mlp_w2, mlp_b2, bnds, Mpad):
    """Build shared weight arrays + per-core input maps."""
    import ml_dtypes
    f8 = ml_dtypes.float8_e4m3
    bf = ml_dtypes.bfloat16

    a = float(np.asarray(prelu_a))
    c_abs = (1.0 - a) / 2.0
    c_lin = (1.0 + a) / 2.0

    # fp8 weights scaled x16 (away from subnormals); unscaled exactly via the
    # bf16 second-layer weights (w2h, m2t) and b1.
    WS = 16.0
    # wh[p, i, s*2+q, m] = [strip(p,i)==s] * WS*gate_w1[q*128+m, p%64]
    wh = np.zeros((P, 2, 8, P), np.float32)
    wm = np.zeros((P, 2, 4, P), np.float32)
    g1 = np.asarray(gate_w1, np.float32)      # [256, 64]
    m1 = np.asarray(mlp_w1, np.float32)       # [128, 64]
    for s in range(4):
        ph, i = s % 2, s // 2
        for q in range(2):
            wh[64 * ph:64 * (ph + 1), i, s * 2 + q, :] = \
                WS * g1[q * P:(q + 1) * P, :].T
        wm[64 * ph:64 * (ph + 1), i, s, :] = WS * m1.T
    w2h = (c_abs / WS * np.asarray(gate_w2, np.float32).T)  # [256, 4]
    w2h = w2h.reshape(2, P, HEADS).transpose(1, 0, 2)  # [128, 2, 4]
    # w12z[p, i, c, h] = [strip(p,i)==c] * c_lin * (gate_w2 @ gate_w1).T[ch, h]
    w12 = c_lin * (np.asarray(gate_w2, np.float32) @ g1).T  # [64, 4]
    w12z = np.zeros((P, 2, 4, HEADS), np.float32)
    for c in range(4):
        ph, i = c % 2, c // 2
        w12z[64 * ph:64 * (ph + 1), i, c, :] = w12

    shared = {
        "wh": np.ascontiguousarray(wh.reshape(P, -1)).astype(f8),
        "wm": np.ascontiguousarray(wm.reshape(P, -1)).astype(f8),
        "w2h": np.ascontiguousarray(w2h.reshape(P, -1)).astype(bf),
        "m2t": np.ascontiguousarray(
            np.asarray(mlp_w2, np.float32).T / WS).astype(bf),
        "b1": np.ascontiguousarray(
            WS * np.asarray(mlp_b1, np.float32).reshape(P, 1)),
        "b2r": np.ascontiguousarray(
            np.tile(np.asarray(mlp_b2, np.float32).reshape(1, P), (P, 1))),
        "w12z": np.ascontiguousarray(w12z.reshape(P, -1)).astype(bf),
    }

    blocks = [BLK_NODES] * (Mpad // BLK_NODES)
    if Mpad % BLK_NODES:
        blocks.append(Mpad % BLK_NODES)
    in_maps = []
    for c in range(len(bnds) - 1):
        r0, r1 = int(bnds[c]), int(bnds[c + 1])
        cnt = r1 - r0
        xs = np.zeros((Mpad, IN_CH), np.float32)
        xs[:cnt] = x[r0:r1]
        x8 = xs.astype(f8)
        onehot = np.zeros((Mpad, P), np.uint8)
        bid = (batch[r0:r1] - c * SEGS_PER_CORE).astype(np.int64)
        onehot[np.arange(cnt), bid] = 1
        # per block: xq[p, i*sb + col] = x[node(b, 2i+p//64, col), p%64]
        #            ind[p, (m, c, s)] = onehot[node(b, c, m*128+p), s]
        xq = np.empty((P, Mpad // 2), f8)
        ind = np.empty((P, Mpad), np.uint8)
        noff = 0
        for bn in blocks:
            sb = bn // 4
            xb = x8[noff:noff + bn].reshape(4, sb, IN_CH)
            for i in range(2):
                for ph in range(2):
                    xq[64 * ph:64 * (ph + 1),
                       noff // 2 + i * sb:noff // 2 + (i + 1) * sb] = \
                        xb[2 * i + ph].T
            ib = onehot[noff:noff + bn].reshape(4, bn // 512, P, P)
            ind[:, noff:noff + bn] = \
                ib.transpose(2, 1, 0, 3).reshape(P, bn)
            noff += bn
        in_maps.append({
            "xq": np.ascontiguousarray(xq),
            "ind": np.ascontiguousarray(ind).astype(f8),
            **shared,
        })
    return in_maps


def kernel(x, batch, num_segments, gate_w1, prelu_a, gate_w2,
           mlp_w1, mlp_b1, mlp_w2, mlp_b2):
    from concourse.bass_utils import run_bass_kernel_spmd

    x = np.asarray(x, dtype=np.float32)
    batch = np.asarray(batch, dtype=np.int32)

    bnds = np.searchsorted(batch, np.arange(0, NUM_SEGS + 1, SEGS_PER_CORE))
    counts = np.diff(bnds)
    Mpad = int(-(-counts.max() // 2048) * 2048)

    nc = _build_bass(Mpad)
    in_maps = _host_inputs(x, batch, gate_w1, prelu_a, gate_w2, mlp_w1,
                           mlp_b1, mlp_w2, mlp_b2, bnds, Mpad)
    res = run_bass_kernel_spmd(nc, in_maps, core_ids=list(range(N_CORES)))
    out = np.concatenate([res.results[c]["out"] for c in range(N_CORES)],
                         axis=0)
    return out.astype(np.float32)
